# revision 47
# baseline (speedup 1.0000x reference)
"""Trainium2 Bass kernel for nn_MMN_7361573945989 (MatchNet corr/attention).

Math (per batch b):
  qn_l = l2norm_c(fq_l); sn_l = l2norm_c(fs_l)           l in {4, 3}
  logits[p, q] = TEMP * (w0 * qn4.T@sn4 + w1 * qn3.T@sn3)[p, q]
  attn = softmax_q(logits)
  att_fq[c, p] = sum_q attn[p, q] * f_s[c, q]
  fq_out = l2norm_c(f_q) + l2norm_c(att_fq) * ATT_WT
  returns (fq_out, att_fq)

Sharding: 8 cores = 2 batches x 4 query-pixel shards of 900.

Per-core kernel (transposed orientation, logits live as [q, p] tiles):
  - all features arrive bf16 from the host; f_s pre-transposed to [hw, cv]
  - query side (P0): squares on DVE (bf16 4x), channel sums via ones-column
    matmuls into [1, PB] PSUM rows, broadcast via K=1 matmul, and the
    inverse norm (scaled by |TEMP*w_l|) comes from one activation:
    Exp(-0.5*Ln(ss) + ln|T*w_l|).  Sign of w_l folds into the query scaling.
  - support side (main loop, per 128-pixel chunk): squares on DVE, then 24
    single-column matmuls (lhsT = squared chunk, rhs = ones) contract the
    channel partition dim directly into per-layer [q, 1] PSUM columns; the
    inverse norms are per-partition [128,1] scalars -- no broadcasts.
  - logits accumulate per layer in separate PSUM groups (ps4 double-
    buffered across chunks, ps3 single), and the softmax exp is split as
    exp(l4 + l3) = Exp(ps4 * inv4[q]) * Exp(ps3 * inv3[q]) using the
    activation's per-partition scale operand; the product is one cheap
    bf16 DVE multiply into the persistent exp table.
  - only Exp/Ln/Square/Copy activations are used -> a single activation
    table ('natural_log_exp_and_others'), no table reloads.
  - phase B: two passes of 2 channel blocks; Y[c,p] accumulates exp @ f_s.T
    over chunks; softmax denominators via ones-matmuls in pass 0; the
    att_fq l2norm uses ||Y|| so the denominator cancels.
"""

import math
import sys
from contextlib import ExitStack

import numpy as np
import ml_dtypes

sys.path.insert(0, "/opt/trn_rl_repo")

import concourse.bass as bass  # noqa: E402
import concourse.tile as tile  # noqa: E402
from concourse import mybir  # noqa: E402
from concourse.bass_utils import run_bass_kernel_spmd  # noqa: E402

B, H, W = 2, 60, 60
HW = H * W  # 3600
C3, C4, CV = 1024, 2048, 512
TEMP = 20.0
ATT_WT = 0.3
NCORES = 8
PSH = 4  # query-pixel shards per batch
P = HW // PSH  # 900 query pixels per core
PB = P // 2  # 450, p-block (one PSUM bank of fp32)
NQC = (HW + 127) // 128  # 29 support-pixel chunks
QT = HW - (NQC - 1) * 128  # 16 rows in the tail chunk
NC4, NC3, NCV = C4 // 128, C3 // 128, CV // 128  # 16, 8, 4
NCI = NC4 + NC3  # 24 combined channel chunks

F32 = mybir.dt.float32
BF16 = mybir.dt.bfloat16
F8 = mybir.dt.float8e4
AF = mybir.ActivationFunctionType
MUL = mybir.AluOpType.mult
DR = mybir.MatmulPerfMode.DoubleRow

_MAX_WAITS_PER_INST = 1


def _patched_drain_and_barrier(self, tick_clock, wait_clock):
    """Tile's kernel-tail drain carries one sem wait per engine/queue; the
    walrus build used here accepts only one sync wait per CTRL instruction.
    Split the waits across extra sync-engine nops."""
    drain_inst = self.nc.sync.drain()
    wait_clock.add_sem_waits(
        drain_inst.ins, tile.ScopedClock({None: tick_clock.global_clock})
    )
    si = drain_inst.ins.sync_info
    if si is not None and len(si.on_wait) > _MAX_WAITS_PER_INST:
        waits = list(si.on_wait)
        drain_inst.ins.sync_info = mybir.SyncInfo(
            on_wait=waits[:_MAX_WAITS_PER_INST], on_update=list(si.on_update)
        )
        for i in range(_MAX_WAITS_PER_INST, len(waits), _MAX_WAITS_PER_INST):
            nop = self.nc.sync.nop()
            nop.ins.sync_info = mybir.SyncInfo(
                on_wait=waits[i : i + _MAX_WAITS_PER_INST], on_update=[]
            )
    self.nc.all_engine_barrier()
    assert self.sems is not None
    popped = self.nc._tile_sem_poison_stack.pop()
    assert popped is self._sem_poison
    self.nc.clear_and_free_semaphores(list(self.sems.allocated().values()))
    self.nc.all_engine_barrier()


tile.TileContext._drain_and_barrier = _patched_drain_and_barrier


def _split_sync_waits(nc, max_waits=_MAX_WAITS_PER_INST):
    """Walrus here accepts at most one sync wait per instruction; move excess
    waits onto same-engine nops inserted immediately before the instruction."""
    ctr = 0
    for f in nc.m.functions:
        for blk in f.blocks:
            insts = list(blk.instructions)
            out = []
            changed = False
            for inst in insts:
                si = inst.sync_info
                if si is not None and len(si.on_wait) > max_waits:
                    waits = list(si.on_wait)
                    for i0 in range(max_waits, len(waits), max_waits):
                        ctr += 1
                        nop = mybir.InstNoOp(
                            name=f"waitsplit-{ctr}",
                            engine=inst.engine,
                            bass_nofuse=True,
                            sync_info=mybir.SyncInfo(
                                on_wait=waits[i0 : i0 + max_waits], on_update=[]
                            ),
                        )
                        nc.register_instruction(nop, overwrite=True)
                        out.append(nop)
                    inst.sync_info = mybir.SyncInfo(
                        on_wait=waits[:max_waits], on_update=list(si.on_update)
                    )
                    changed = True
                out.append(inst)
            if changed:
                blk.instructions = out


def build():
    nc = bass.Bass()
    q4 = nc.dram_tensor("q4", [C4, P], BF16, kind="ExternalInput")
    q3 = nc.dram_tensor("q3", [C3, P], BF16, kind="ExternalInput")
    s4 = nc.dram_tensor("s4", [C4, HW], F8, kind="ExternalInput")
    s3 = nc.dram_tensor("s3", [C3, HW], BF16, kind="ExternalInput")
    vt = nc.dram_tensor("vt", [HW, CV], BF16, kind="ExternalInput")  # f_s.T
    fq = nc.dram_tensor("fq", [CV, P], BF16, kind="ExternalInput")
    # wv = [[ln|T*w0|, ln|T*w1|]]
    wv = nc.dram_tensor("wv", [1, 2], F32, kind="ExternalInput")
    att_o = nc.dram_tensor("att_o", [CV, P], F32, kind="ExternalOutput")
    fq_o = nc.dram_tensor("fq_o", [CV, P], F32, kind="ExternalOutput")

    def load_blocks(dst, dst_cols, ci0, src, col0, ncols, n_ci, group=8, eng=None):
        """Load `n_ci` row-blocks of 128 from DRAM `src` (cols [col0,col0+ncols))
        into SBUF tile `dst` whose free layout is (ci, dst_cols)."""
        eng = eng or nc.sync
        srcr = src[:].rearrange("(ci c) x -> c ci x", c=128)
        dstr = dst[:].rearrange("c (ci x) -> c ci x", x=dst_cols)
        for g0 in range(0, n_ci, group):
            g = min(group, n_ci - g0)
            eng.dma_start(
                dstr[:, ci0 + g0 : ci0 + g0 + g, 0:ncols],
                srcr[:, g0 : g0 + g, col0 : col0 + ncols],
            )

    with tile.TileContext(nc) as tc:
        with ExitStack() as octx:
            cpool = octx.enter_context(tc.tile_pool(name="const", bufs=1))
            ones_col = cpool.tile([128, 1], BF16)
            nc.gpsimd.memset(ones_col[:], 1.0)
            ones_row = cpool.tile([1, 128], F32)
            nc.gpsimd.memset(ones_row[:], 1.0)
            w_sb = cpool.tile([1, 2], F32)
            nc.sync.dma_start(w_sb[:], wv[:])
            w_col = cpool.tile([128, 2], F32)
            lnw_att = cpool.tile([128, 1], F32)
            nc.gpsimd.memset(lnw_att[:], float(math.log(ATT_WT)))

            pers = octx.enter_context(tc.tile_pool(name="pers", bufs=1))
            qns4 = pers.tile([128, NC4 * P], F8)  # scaled query l4 (ci, p)
            qns3 = pers.tile([128, NC3 * P], BF16)  # scaled query l3 (ci, p)
            expT = pers.tile([128, NQC * P], BF16)  # exp(logits) (qc; q, p)
            vt_all = pers.tile([128, NQC * CV], BF16)  # f_s.T chunks (qc; q, c)
            # zero the tail-chunk regions so K=128 matmuls over the tail are
            # exact (rows [0:QT] get real data later)
            nc.gpsimd.memset(expT[:, (NQC - 1) * P : NQC * P], 0.0)
            nc.gpsimd.memset(vt_all[:, (NQC - 1) * CV : NQC * CV], 0.0)

            # broadcast ln|T*w| across partitions once: [1,2] -> [128,2]
            with tc.tile_pool(name="wps", bufs=1, space="PSUM") as wps:
                w_ps = wps.tile([128, 2], F32)
                nc.tensor.matmul(w_ps[:], ones_row[:], w_sb[:])
                nc.scalar.copy(w_col[:], w_ps[:])

            # main-loop pools are created BEFORE P0's so their SBUF does not
            # alias P0's staging tiles (aliasing would chain the support
            # DMAs behind P0's long DVE tail)
            mctx = octx.enter_context(ExitStack())
            snpool = mctx.enter_context(tc.tile_pool(name="sn", bufs=3))
            sqpool = mctx.enter_context(tc.tile_pool(name="msq", bufs=2))
            mmini = mctx.enter_context(tc.tile_pool(name="mmini", bufs=2))
            epool = mctx.enter_context(tc.tile_pool(name="etmp", bufs=2))
            mps = mctx.enter_context(tc.tile_pool(name="mps", bufs=1, space="PSUM"))

            # ---------------- P0: query-side normalization ----------------
            # raw tile (bf16), dst tile, n_ci, dram src, w-col index
            # (None => no |T*w|); sign(w_l) is folded on the host.
            # ss_tags/bc_tag choose which PSUM banks to borrow.
            def q_norm_layer(
                sqp, mini, pps, li, raw, dst, n_ci, src, wl,
                ss_tags=None, ss_bufs=None, bc_tag="bc", bc_bufs=2,
            ):
                load_blocks(raw, P, 0, src, 0, P, n_ci, group=2)
                ss = [
                    pps.tile(
                        [1, PB], F32,
                        tag=ss_tags[pb] if ss_tags else f"ss{li}_{pb}",
                        name=f"ssq{li}_{pb}",
                        bufs=ss_bufs,
                    )
                    for pb in range(2)
                ]
                for k in range(n_ci):
                    xg = raw[:, k * P : (k + 1) * P]
                    sq = sqp.tile([128, P], BF16, tag="sq", name=f"sqq{li}_{k}")
                    nc.vector.tensor_mul(sq[:], xg, xg)
                    for pb in range(2):
                        nc.tensor.matmul(
                            ss[pb][:],
                            ones_col[:],
                            sq[:, pb * PB : (pb + 1) * PB],
                            start=(k == 0),
                            stop=(k == n_ci - 1),
                        )
                invqs = []
                for pb in range(2):
                    u = mini.tile([1, PB], F32, tag="u", name=f"u{li}_{pb}")
                    nc.scalar.copy(u[:], ss[pb][:])
                    bc = pps.tile(
                        [128, PB], F32, tag=bc_tag, name=f"bc{li}_{pb}",
                        bufs=bc_bufs,
                    )
                    nc.tensor.matmul(bc[:], ones_row[:], u[:])
                    lnb = mini.tile([128, PB], F32, tag="lnb", name=f"lnb{li}_{pb}")
                    nc.scalar.activation(lnb[:], bc[:], AF.Ln)
                    invq = mini.tile(
                        [128, PB], BF16, tag="invq", name=f"invq{li}_{pb}"
                    )
                    # Exp(-0.5*ln(ss) + ln|T*w|) = |T*w| / sqrt(ss)
                    if wl is None:
                        nc.scalar.activation(invq[:], lnb[:], AF.Exp, scale=-0.5)
                    else:
                        nc.scalar.activation(
                            invq[:],
                            lnb[:],
                            AF.Exp,
                            scale=-0.5,
                            bias=w_col[:, wl : wl + 1],
                        )
                    invqs.append(invq)
                # k-outer so the first channel chunks are ready for the
                # main loop's first matmuls as early as possible
                for k in range(n_ci):
                    for pb in range(2):
                        sl = slice(k * P + pb * PB, k * P + pb * PB + PB)
                        nc.vector.tensor_mul(
                            dst[:, sl], raw[:, sl], invqs[pb][:]
                        )

            # layer 3 first: its chain is short, so the main loop's layer-3
            # matmuls (emitted first per chunk) start earliest; f_q is
            # normalized after the main loop (it is only needed in phase B).
            # P0 borrows the main PSUM banks that the main loop touches
            # late (ps4 via back(), ss4): the rotation deps are harmless.
            with ExitStack() as pctx:
                sqp = pctx.enter_context(tc.tile_pool(name="p0sq", bufs=2))
                mini = pctx.enter_context(tc.tile_pool(name="p0mini", bufs=2))
                stg = pctx.enter_context(tc.tile_pool(name="p0stg", bufs=1))
                q4s = stg.tile([128, NC4 * P], BF16)  # raw q4 staging
                q_norm_layer(
                    sqp, mini, mps, 0, qns3, qns3, NC3, q3, 1,
                    ss_tags=["ps4_0", "ps4_1"], ss_bufs=2,
                    bc_tag="ss4", bc_bufs=1,
                )
                q_norm_layer(
                    sqp, mini, mps, 1, q4s, qns4, NC4, q4, 0,
                    ss_tags=["ps4_0", "ps4_1"], ss_bufs=2,
                    bc_tag="ss4", bc_bufs=1,
                )

            # ------------- main: support stream, logits, exp -------------
            # Split per chunk into a front half (layer-3 logits, support
            # norms, exp3) and a back half (layer-4 logits, exp4, product).
            # Fronts run WARM chunks ahead so the PE has queued work while
            # P0's layer-4 scaling (a long DVE chain) finishes.
            WARM = 4
            NSCAL = 6  # chunks whose squares run on the scalar engine
            if True:
                qns4r = qns4[:].rearrange("c (ci p) -> c ci p", p=P)
                state = {}

                def support_inv(qc, qn, tag, sq_t, n_ci, itag, ibufs):
                    """1/||s_q|| as a [q, 1] column: ones-column stays the PE
                    weights while the squares stream (one LDWEIGHTS for the
                    whole sum), row Ln/Exp, then a K=1 matmul transposes the
                    row back onto partitions (same PSUM bank, rotated)."""
                    ssr = mps.tile([1, 128], F32, tag=tag, name=f"{tag}r_{qc}")
                    for ci in range(n_ci):
                        nc.tensor.matmul(
                            ssr[0:1, 0:qn],
                            ones_col[:],
                            sq_t[:, ci * 128 : ci * 128 + qn],
                            start=(ci == 0),
                            stop=(ci == n_ci - 1),
                        )
                    lnr = mmini.tile([1, 128], F32, tag=f"ln{tag}", name=f"ln{tag}_{qc}")
                    nc.scalar.activation(lnr[0:1, 0:qn], ssr[0:1, 0:qn], AF.Ln)
                    invr = mmini.tile([1, 128], F32, tag=f"iv{tag}", name=f"iv{tag}_{qc}")
                    nc.scalar.activation(
                        invr[0:1, 0:qn], lnr[0:1, 0:qn], AF.Exp, scale=-0.5
                    )
                    col = mps.tile([128, 1], F32, tag=tag, name=f"{tag}c_{qc}")
                    nc.tensor.matmul(
                        col[0:qn, :], invr[0:1, 0:qn], ones_row[:, 0:1]
                    )
                    inv = mmini.tile(
                        [128, 1], F32, tag=itag, name=f"{itag}_{qc}", bufs=ibufs
                    )
                    nc.scalar.copy(inv[0:qn, :], col[0:qn, :])
                    return inv

                def front(qc):
                    qn = 128 if qc < NQC - 1 else QT
                    sn3 = snpool.tile(
                        [128, NC3 * 128], BF16, tag="sn3", name=f"sn3_{qc}"
                    )
                    sn4 = snpool.tile(
                        [128, NC4 * 128], F8, tag="sn4", name=f"sn4_{qc}",
                        bufs=WARM + 1,
                    )
                    load_blocks(sn3, 128, 0, s3, qc * 128, qn, NC3, eng=nc.gpsimd)
                    load_blocks(sn4, 128, 0, s4, qc * 128, qn, NC4, eng=nc.gpsimd)
                    nc.gpsimd.dma_start(
                        vt_all[0:qn, qc * CV : (qc + 1) * CV],
                        vt[qc * 128 : qc * 128 + qn, :],
                    )
                    sq3 = sqpool.tile(
                        [128, NC3 * 128], BF16, tag="sq3", name=f"sq3_{qc}"
                    )
                    if qc < NSCAL:
                        # scalar-engine square: the early chunks' layer-3
                        # norm chain must not queue behind P0's DVE work
                        nc.scalar.square(sq3[:], sn3[:])
                    else:
                        nc.vector.tensor_mul(sq3[:], sn3[:], sn3[:])
                    # sq4 is only needed in back(qc), WARM chunks later, so
                    # the DVE always has slack for it
                    sq4 = sqpool.tile(
                        [128, NC4 * 128], BF16, tag="sq4", name=f"sq4_{qc}",
                        bufs=WARM + 1,
                    )
                    nc.vector.tensor_mul(sq4[:], sn4[:], sn4[:])

                    ps3 = [
                        mps.tile(
                            [128, PB], F32, tag=f"ps3_{pb}", name=f"ps3_{pb}_{qc}"
                        )
                        for pb in range(2)
                    ]
                    for k in range(NC3):
                        lhsT = sn3[:, k * 128 : k * 128 + qn]
                        for pb in range(2):
                            nc.tensor.matmul(
                                ps3[pb][0:qn, :],
                                lhsT,
                                qns3[:, k * P + pb * PB : k * P + pb * PB + PB],
                                start=(k == 0),
                                stop=(k == NC3 - 1),
                            )
                    inv3 = support_inv(qc, qn, "ss3", sq3, NC3, "inv3", 2)
                    e3s = []
                    for pb in range(2):
                        e3 = epool.tile(
                            [128, PB], BF16, tag=f"e3_{pb}", name=f"e3_{pb}_{qc}",
                            bufs=WARM + 1,
                        )
                        nc.scalar.activation(
                            e3[0:qn, :],
                            ps3[pb][0:qn, :],
                            AF.Exp,
                            scale=inv3[0:qn, :],
                        )
                        e3s.append(e3)
                    state[qc] = (qn, sn4, sq4, e3s)

                def back(qc):
                    qn, sn4, sq4, e3s = state.pop(qc)
                    ps4 = [
                        mps.tile(
                            [128, PB], F32, tag=f"ps4_{pb}", name=f"ps4_{pb}_{qc}",
                            bufs=2,
                        )
                        for pb in range(2)
                    ]
                    sn4r = sn4[:].rearrange("c (ci q) -> c ci q", q=128)
                    for i in range(NC4 // 2):
                        lhsT = sn4r[:, 2 * i : 2 * i + 2, 0:qn]
                        for pb in range(2):
                            nc.tensor.matmul(
                                ps4[pb][0:qn, :],
                                lhsT,
                                qns4r[
                                    :, 2 * i : 2 * i + 2,
                                    pb * PB : pb * PB + PB,
                                ],
                                start=(i == 0),
                                stop=(i == NC4 // 2 - 1),
                                perf_mode=DR,
                            )
                    inv4 = support_inv(qc, qn, "ss4", sq4, NC4, "inv4", 2)
                    for pb in range(2):
                        e4 = epool.tile(
                            [128, PB], BF16, tag=f"e4_{pb}", name=f"e4_{pb}_{qc}"
                        )
                        nc.scalar.activation(
                            e4[0:qn, :],
                            ps4[pb][0:qn, :],
                            AF.Exp,
                            scale=inv4[0:qn, :],
                        )
                        nc.vector.tensor_mul(
                            expT[0:qn, qc * P + pb * PB : qc * P + pb * PB + PB],
                            e4[0:qn, :],
                            e3s[pb][0:qn, :],
                        )

                # warmup: fronts for the first WARM chunks fill the PE while
                # P0's layer-4 scaling finishes; afterwards chunks run
                # monolithically (front+back) so the steady state has a
                # single wait point per iteration (protects the p-state ramp)
                for qc in range(WARM):
                    front(qc)
                for qc in range(WARM):
                    back(qc)
                for qc in range(WARM, NQC):
                    front(qc)
                    back(qc)
            mctx.close()

            fstg = octx.enter_context(tc.tile_pool(name="fstg", bufs=1))
            fqn = fstg.tile([128, NCV * P], BF16)  # normalized f_q (ci, p)

            # ---------------- phase B: attention-weighted values ----------------
            with ExitStack() as bctx:
                bps = bctx.enter_context(
                    tc.tile_pool(name="bps", bufs=1, space="PSUM")
                )
                bmini = bctx.enter_context(tc.tile_pool(name="bmini", bufs=1))
                bsq = bctx.enter_context(tc.tile_pool(name="bsq", bufs=2))
                batt = bctx.enter_context(tc.tile_pool(name="batt", bufs=1))
                bout = bctx.enter_context(tc.tile_pool(name="bout", bufs=2))

                ssy = [
                    bps.tile([1, PB], F32, tag=f"ssy{pb}", name=f"ssy{pb}")
                    for pb in range(2)
                ]
                dns = [
                    bps.tile([1, PB], F32, tag=f"dn{pb}", name=f"dn{pb}")
                    for pb in range(2)
                ]
                att_sb = {}
                inv_dn, raw_dn = [], []
                # denominators first: their matmuls keep the PE busy while
                # the f_q normalization chain (below) runs on DVE/scalar
                for qc in range(NQC):
                    for pb in range(2):
                        nc.tensor.matmul(
                            dns[pb][:],
                            ones_col[:],
                            expT[:, qc * P + pb * PB : qc * P + pb * PB + PB],
                            start=(qc == 0),
                            stop=(qc == NQC - 1),
                        )
                for pb in range(2):
                    u = bmini.tile([1, PB], F32, tag=f"ud{pb}", name=f"ud{pb}")
                    nc.scalar.copy(u[:], dns[pb][:])
                    bcp = bps.tile([128, PB], F32, tag=f"dn{pb}", name=f"bd{pb}")
                    nc.tensor.matmul(bcp[:], ones_row[:], u[:])
                    raw = bmini.tile(
                        [128, PB], F32, tag=f"dnraw{pb}", name=f"dnraw{pb}"
                    )
                    nc.scalar.copy(raw[:], bcp[:])
                    inv = bmini.tile(
                        [128, PB], F32, tag=f"dninv{pb}", name=f"dninv{pb}"
                    )
                    nc.vector.reciprocal(inv[:], bcp[:])
                    inv_dn.append(inv)
                    raw_dn.append(raw)

                # f_q normalization: deferred to here (only the phase-B
                # epilogue reads fqn) so its DVE/scalar chain hides under
                # the dn/Y matmuls; borrows the y0 PSUM banks (their first
                # real use comes later, the rotation deps are harmless)
                with ExitStack() as fctx:
                    fsqp = fctx.enter_context(tc.tile_pool(name="fsq", bufs=2))
                    fmini = fctx.enter_context(tc.tile_pool(name="fmini", bufs=2))
                    q_norm_layer(
                        fsqp, fmini, bps, 2, fqn, fqn, NCV, fq, None,
                        ss_tags=["y0_0", "y0_1"], ss_bufs=1,
                        bc_tag="y1_0", bc_bufs=1,
                    )
                for pss in range(2):
                    ys = {}
                    for cbk in range(2):
                        cb = 2 * pss + cbk
                        for pb in range(2):
                            ys[(cb, pb)] = bps.tile(
                                [128, PB], F32, tag=f"y{cbk}_{pb}",
                                name=f"y{cb}_{pb}",
                            )
                    for qc in range(NQC):
                        for cbk in range(2):
                            cb = 2 * pss + cbk
                            lhsT = vt_all[
                                :, qc * CV + cb * 128 : qc * CV + (cb + 1) * 128
                            ]
                            for pb in range(2):
                                nc.tensor.matmul(
                                    ys[(cb, pb)][:],
                                    lhsT,
                                    expT[:, qc * P + pb * PB : qc * P + pb * PB + PB],
                                    start=(qc == 0),
                                    stop=(qc == NQC - 1),
                                )
                    # pb-major drains: ssy[pb0] stops as early as possible so
                    # the pb0 epilogue overlaps the pb1 drains of pass 1
                    for pb in range(2):
                        for cbk in range(2):
                            cb = 2 * pss + cbk
                            att = batt.tile(
                                [128, PB], F32, tag=f"att{cb}_{pb}",
                                name=f"att{cb}_{pb}",
                            )
                            nc.vector.tensor_mul(
                                att[:], ys[(cb, pb)][:], inv_dn[pb][:]
                            )
                            att_sb[(cb, pb)] = att
                            nc.sync.dma_start(
                                att_o[
                                    cb * 128 : (cb + 1) * 128,
                                    pb * PB : (pb + 1) * PB,
                                ],
                                att[:],
                            )
                            sqy = bsq.tile([128, PB], BF16, tag="sqy")
                            nc.scalar.square(sqy[:], ys[(cb, pb)][:])
                            nc.tensor.matmul(
                                ssy[pb][:],
                                ones_col[:],
                                sqy[:],
                                start=(cb == 0),
                                stop=(cb == NCV - 1),
                            )
                        if pss == 1:
                            # per-pb epilogue right after ssy[pb] stops
                            u = bmini.tile(
                                [1, PB], F32, tag=f"us{pb}", name=f"us{pb}"
                            )
                            nc.scalar.copy(u[:], ssy[pb][:])
                            bcp = bps.tile(
                                [128, PB], F32, tag=f"ssy{pb}", name=f"bs{pb}"
                            )
                            nc.tensor.matmul(bcp[:], ones_row[:], u[:])
                            lnb = bmini.tile(
                                [128, PB], F32, tag=f"lnbs{pb}", name=f"lnbs{pb}"
                            )
                            nc.scalar.activation(lnb[:], bcp[:], AF.Ln)
                            # Exp(-0.5*ln(ssy) + ln(0.3)) = 0.3/||Y||
                            sinv = bmini.tile(
                                [128, PB], F32, tag=f"sinv{pb}", name=f"sinv{pb}"
                            )
                            nc.scalar.activation(
                                sinv[:], lnb[:], AF.Exp, scale=-0.5,
                                bias=lnw_att[:],
                            )
                            # fq = fqn + att * (denom * 0.3/||Y||)
                            s2 = bmini.tile(
                                [128, PB], F32, tag=f"s2{pb}", name=f"s2{pb}"
                            )
                            nc.vector.tensor_mul(s2[:], raw_dn[pb][:], sinv[:])
                            for cb in range(NCV):
                                t = bout.tile([128, PB], F32, tag="t")
                                nc.vector.tensor_mul(
                                    t[:], att_sb[(cb, pb)][:], s2[:]
                                )
                                f_sb = bout.tile([128, PB], F32, tag="f")
                                nc.vector.tensor_add(
                                    f_sb[:],
                                    t[:],
                                    fqn[
                                        :,
                                        cb * P + pb * PB : cb * P + pb * PB + PB,
                                    ],
                                )
                                # fq_o rides the gpsimd DMA queue so the two
                                # output streams drain in parallel
                                nc.gpsimd.dma_start(
                                    fq_o[
                                        cb * 128 : (cb + 1) * 128,
                                        pb * PB : (pb + 1) * PB,
                                    ],
                                    f_sb[:],
                                )
    _split_sync_waits(nc)
    return nc


def make_in_maps(fq_l3, fs_l3, fq_l4, fs_l4, f_q, f_s, w_red):
    bf = ml_dtypes.bfloat16
    wr = np.asarray(w_red, np.float32)
    wvec = np.log(np.abs(TEMP * wr)).reshape(1, 2)
    # fold sign(w_l) into the query features; |T*w_l| rides in wvec
    q4f = float(np.sign(wr[0])) * np.asarray(fq_l4, np.float32).reshape(B, C4, HW)
    q3f = float(np.sign(wr[1])) * np.asarray(fq_l3, np.float32).reshape(B, C3, HW)
    s4f = np.asarray(fs_l4, np.float32).reshape(B, C4, HW)
    s3f = np.asarray(fs_l3, np.float32).reshape(B, C3, HW)
    vf = np.asarray(f_s, np.float32).reshape(B, CV, HW)
    fqf = np.asarray(f_q, np.float32).reshape(B, CV, HW)
    f8 = ml_dtypes.float8_e4m3
    s4b = [np.ascontiguousarray(s4f[b]).astype(bf) for b in range(B)]
    s48 = [x.astype(f8) for x in s4b]
    s3b = [np.ascontiguousarray(s3f[b]).astype(bf) for b in range(B)]
    vtb = [np.ascontiguousarray(vf[b].T).astype(bf) for b in range(B)]
    in_maps = []
    for k in range(NCORES):
        b, j = divmod(k, PSH)
        sl = slice(j * P, (j + 1) * P)
        in_maps.append(
            {
                "q4": np.ascontiguousarray(q4f[b][:, sl]).astype(bf),
                "q3": np.ascontiguousarray(q3f[b][:, sl]).astype(bf),
                "s4": s48[b],
                "s3": s3b[b],
                "vt": vtb[b],
                "fq": np.ascontiguousarray(fqf[b][:, sl]).astype(bf),
                "wv": np.ascontiguousarray(wvec, np.float32),
            }
        )
    return in_maps


def gather_outputs(results):
    att = np.empty((B, CV, HW), np.float32)
    fqo = np.empty((B, CV, HW), np.float32)
    for k in range(NCORES):
        b, j = divmod(k, PSH)
        sl = slice(j * P, (j + 1) * P)
        att[b][:, sl] = results[k]["att_o"]
        fqo[b][:, sl] = results[k]["fq_o"]
    return (
        fqo.reshape(B, CV, H, W),
        att.reshape(B, CV, H, W),
    )


def kernel(fq_l3, fs_l3, fq_l4, fs_l4, f_q, f_s, w_red, trace=False):
    nc = build()
    in_maps = make_in_maps(fq_l3, fs_l3, fq_l4, fs_l4, f_q, f_s, w_red)
    res = run_bass_kernel_spmd(nc, in_maps, core_ids=list(range(NCORES)), trace=trace)
    out = gather_outputs(res.results)
    if trace:
        return out, res
    return out


# revision 53
# speedup vs baseline: 1.0758x; 1.0758x over previous
"""Trainium2 Bass kernel for nn_MMN_7361573945989 (MatchNet corr/attention).

Math (per batch b):
  qn_l = l2norm_c(fq_l); sn_l = l2norm_c(fs_l)           l in {4, 3}
  logits[p, q] = TEMP * (w0 * qn4.T@sn4 + w1 * qn3.T@sn3)[p, q]
  attn = softmax_q(logits)
  att_fq[c, p] = sum_q attn[p, q] * f_s[c, q]
  fq_out = l2norm_c(f_q) + l2norm_c(att_fq) * ATT_WT
  returns (fq_out, att_fq)

Sharding: 8 cores = 2 batches x 4 query-pixel shards of 900.

Per-core kernel (transposed orientation, logits live as [q, p] tiles):
  - all features arrive bf16 from the host; f_s pre-transposed to [hw, cv]
  - query side (P0): squares on DVE (bf16 4x), channel sums via ones-column
    matmuls into [1, PB] PSUM rows, broadcast via K=1 matmul, and the
    inverse norm (scaled by |TEMP*w_l|) comes from one activation:
    Exp(-0.5*Ln(ss) + ln|T*w_l|).  Sign of w_l folds into the query scaling.
  - support side (main loop, per 128-pixel chunk): squares on DVE, then 24
    single-column matmuls (lhsT = squared chunk, rhs = ones) contract the
    channel partition dim directly into per-layer [q, 1] PSUM columns; the
    inverse norms are per-partition [128,1] scalars -- no broadcasts.
  - logits accumulate per layer in separate PSUM groups (ps4 double-
    buffered across chunks, ps3 single), and the softmax exp is split as
    exp(l4 + l3) = Exp(ps4 * inv4[q]) * Exp(ps3 * inv3[q]) using the
    activation's per-partition scale operand; the product is one cheap
    bf16 DVE multiply into the persistent exp table.
  - only Exp/Ln/Square/Copy activations are used -> a single activation
    table ('natural_log_exp_and_others'), no table reloads.
  - phase B: two passes of 2 channel blocks; Y[c,p] accumulates exp @ f_s.T
    over chunks; softmax denominators via ones-matmuls in pass 0; the
    att_fq l2norm uses ||Y|| so the denominator cancels.
"""

import math
import sys
from contextlib import ExitStack

import numpy as np
import ml_dtypes

sys.path.insert(0, "/opt/trn_rl_repo")

import concourse.bass as bass  # noqa: E402
import concourse.tile as tile  # noqa: E402
from concourse import mybir  # noqa: E402
from concourse.bass_utils import run_bass_kernel_spmd  # noqa: E402

B, H, W = 2, 60, 60
HW = H * W  # 3600
C3, C4, CV = 1024, 2048, 512
TEMP = 20.0
ATT_WT = 0.3
NCORES = 8
PSH = 4  # query-pixel shards per batch
P = HW // PSH  # 900 query pixels per core
PB = P // 2  # 450, p-block (one PSUM bank of fp32)
NQC = (HW + 127) // 128  # 29 support-pixel chunks
QT = HW - (NQC - 1) * 128  # 16 rows in the tail chunk
NC4, NC3, NCV = C4 // 128, C3 // 128, CV // 128  # 16, 8, 4
NCI = NC4 + NC3  # 24 combined channel chunks

F32 = mybir.dt.float32
BF16 = mybir.dt.bfloat16
F8 = mybir.dt.float8e4
AF = mybir.ActivationFunctionType
MUL = mybir.AluOpType.mult
DR = mybir.MatmulPerfMode.DoubleRow

_MAX_WAITS_PER_INST = 1


def _patched_drain_and_barrier(self, tick_clock, wait_clock):
    """Tile's kernel-tail drain carries one sem wait per engine/queue; the
    walrus build used here accepts only one sync wait per CTRL instruction.
    Split the waits across extra sync-engine nops."""
    drain_inst = self.nc.sync.drain()
    wait_clock.add_sem_waits(
        drain_inst.ins, tile.ScopedClock({None: tick_clock.global_clock})
    )
    si = drain_inst.ins.sync_info
    if si is not None and len(si.on_wait) > _MAX_WAITS_PER_INST:
        waits = list(si.on_wait)
        drain_inst.ins.sync_info = mybir.SyncInfo(
            on_wait=waits[:_MAX_WAITS_PER_INST], on_update=list(si.on_update)
        )
        for i in range(_MAX_WAITS_PER_INST, len(waits), _MAX_WAITS_PER_INST):
            nop = self.nc.sync.nop()
            nop.ins.sync_info = mybir.SyncInfo(
                on_wait=waits[i : i + _MAX_WAITS_PER_INST], on_update=[]
            )
    self.nc.all_engine_barrier()
    assert self.sems is not None
    popped = self.nc._tile_sem_poison_stack.pop()
    assert popped is self._sem_poison
    self.nc.clear_and_free_semaphores(list(self.sems.allocated().values()))
    self.nc.all_engine_barrier()


tile.TileContext._drain_and_barrier = _patched_drain_and_barrier


def _split_sync_waits(nc, max_waits=_MAX_WAITS_PER_INST):
    """Walrus here accepts at most one sync wait per instruction; move excess
    waits onto same-engine nops inserted immediately before the instruction."""
    ctr = 0
    for f in nc.m.functions:
        for blk in f.blocks:
            insts = list(blk.instructions)
            out = []
            changed = False
            for inst in insts:
                si = inst.sync_info
                if si is not None and len(si.on_wait) > max_waits:
                    waits = list(si.on_wait)
                    for i0 in range(max_waits, len(waits), max_waits):
                        ctr += 1
                        nop = mybir.InstNoOp(
                            name=f"waitsplit-{ctr}",
                            engine=inst.engine,
                            bass_nofuse=True,
                            sync_info=mybir.SyncInfo(
                                on_wait=waits[i0 : i0 + max_waits], on_update=[]
                            ),
                        )
                        nc.register_instruction(nop, overwrite=True)
                        out.append(nop)
                    inst.sync_info = mybir.SyncInfo(
                        on_wait=waits[:max_waits], on_update=list(si.on_update)
                    )
                    changed = True
                out.append(inst)
            if changed:
                blk.instructions = out


def build():
    nc = bass.Bass()
    q4 = nc.dram_tensor("q4", [C4, P], BF16, kind="ExternalInput")
    q3 = nc.dram_tensor("q3", [C3, P], BF16, kind="ExternalInput")
    s4 = nc.dram_tensor("s4", [C4, HW], F8, kind="ExternalInput")
    s3 = nc.dram_tensor("s3", [C3, HW], BF16, kind="ExternalInput")
    vt = nc.dram_tensor("vt", [HW, CV], BF16, kind="ExternalInput")  # f_s.T
    fq = nc.dram_tensor("fq", [CV, P], BF16, kind="ExternalInput")
    # wv = [[ln|T*w0|, ln|T*w1|]]
    wv = nc.dram_tensor("wv", [1, 2], F32, kind="ExternalInput")
    att_o = nc.dram_tensor("att_o", [CV, P], F32, kind="ExternalOutput")
    fq_o = nc.dram_tensor("fq_o", [CV, P], F32, kind="ExternalOutput")

    def load_blocks(dst, dst_cols, ci0, src, col0, ncols, n_ci, group=8, eng=None):
        """Load `n_ci` row-blocks of 128 from DRAM `src` (cols [col0,col0+ncols))
        into SBUF tile `dst` whose free layout is (ci, dst_cols)."""
        eng = eng or nc.sync
        srcr = src[:].rearrange("(ci c) x -> c ci x", c=128)
        dstr = dst[:].rearrange("c (ci x) -> c ci x", x=dst_cols)
        for g0 in range(0, n_ci, group):
            g = min(group, n_ci - g0)
            eng.dma_start(
                dstr[:, ci0 + g0 : ci0 + g0 + g, 0:ncols],
                srcr[:, g0 : g0 + g, col0 : col0 + ncols],
            )

    with tile.TileContext(nc) as tc:
        with ExitStack() as octx:
            cpool = octx.enter_context(tc.tile_pool(name="const", bufs=1))
            ones_col = cpool.tile([128, 1], BF16)
            nc.gpsimd.memset(ones_col[:], 1.0)
            # k-tile pair of 32-wide ones blocks (DoubleRow LDWEIGHTS
            # rejects single-column weights; rows 0..31 of the output all
            # carry the same column sum and row 0 is read)
            ones8 = cpool.tile([128, 64], F8)
            nc.gpsimd.memset(ones8[:], 1.0)
            ones_row = cpool.tile([1, 128], F32)
            nc.gpsimd.memset(ones_row[:], 1.0)
            w_sb = cpool.tile([1, 2], F32)
            nc.sync.dma_start(w_sb[:], wv[:])
            w_col = cpool.tile([128, 2], F32)
            lnw_att = cpool.tile([128, 1], F32)
            nc.gpsimd.memset(lnw_att[:], float(math.log(ATT_WT)))

            pers = octx.enter_context(tc.tile_pool(name="pers", bufs=1))
            qns4 = pers.tile([128, NC4 * P], F8)  # scaled query l4 (ci, p)
            qns3 = pers.tile([128, NC3 * P], BF16)  # scaled query l3 (ci, p)
            expT = pers.tile([128, NQC * P], BF16)  # exp(logits) (qc; q, p)
            vt_all = pers.tile([128, NQC * CV], BF16)  # f_s.T chunks (qc; q, c)
            # zero the tail-chunk regions so K=128 matmuls over the tail are
            # exact (rows [0:QT] get real data later)
            nc.gpsimd.memset(expT[:, (NQC - 1) * P : NQC * P], 0.0)
            nc.gpsimd.memset(vt_all[:, (NQC - 1) * CV : NQC * CV], 0.0)

            # broadcast ln|T*w| across partitions once: [1,2] -> [128,2]
            with tc.tile_pool(name="wps", bufs=1, space="PSUM") as wps:
                w_ps = wps.tile([128, 2], F32)
                nc.tensor.matmul(w_ps[:], ones_row[:], w_sb[:])
                nc.scalar.copy(w_col[:], w_ps[:])

            # main-loop pools are created BEFORE P0's so their SBUF does not
            # alias P0's staging tiles (aliasing would chain the support
            # DMAs behind P0's long DVE tail)
            mctx = octx.enter_context(ExitStack())
            snpool = mctx.enter_context(tc.tile_pool(name="sn", bufs=3))
            sqpool = mctx.enter_context(tc.tile_pool(name="msq", bufs=2))
            mmini = mctx.enter_context(tc.tile_pool(name="mmini", bufs=2))
            epool = mctx.enter_context(tc.tile_pool(name="etmp", bufs=2))
            mps = mctx.enter_context(tc.tile_pool(name="mps", bufs=1, space="PSUM"))

            # ---------------- P0: query-side normalization ----------------
            # raw tile (bf16), dst tile, n_ci, dram src, w-col index
            # (None => no |T*w|); sign(w_l) is folded on the host.
            # ss_tags/bc_tag choose which PSUM banks to borrow.
            def q_norm_layer(
                sqp, mini, pps, li, raw, dst, n_ci, src, wl,
                ss_tags=None, ss_bufs=None, bc_tag="bc", bc_bufs=2,
            ):
                load_blocks(raw, P, 0, src, 0, P, n_ci, group=2)
                ss = [
                    pps.tile(
                        [1, PB], F32,
                        tag=ss_tags[pb] if ss_tags else f"ss{li}_{pb}",
                        name=f"ssq{li}_{pb}",
                        bufs=ss_bufs,
                    )
                    for pb in range(2)
                ]
                for k in range(n_ci):
                    xg = raw[:, k * P : (k + 1) * P]
                    sq = sqp.tile([128, P], BF16, tag="sq", name=f"sqq{li}_{k}")
                    nc.vector.tensor_mul(sq[:], xg, xg)
                    for pb in range(2):
                        nc.tensor.matmul(
                            ss[pb][:],
                            ones_col[:],
                            sq[:, pb * PB : (pb + 1) * PB],
                            start=(k == 0),
                            stop=(k == n_ci - 1),
                        )
                invqs = []
                for pb in range(2):
                    u = mini.tile([1, PB], F32, tag="u", name=f"u{li}_{pb}")
                    nc.scalar.copy(u[:], ss[pb][:])
                    bc = pps.tile(
                        [128, PB], F32, tag=bc_tag, name=f"bc{li}_{pb}",
                        bufs=bc_bufs,
                    )
                    nc.tensor.matmul(bc[:], ones_row[:], u[:])
                    lnb = mini.tile([128, PB], F32, tag="lnb", name=f"lnb{li}_{pb}")
                    nc.scalar.activation(lnb[:], bc[:], AF.Ln)
                    invq = mini.tile(
                        [128, PB], BF16, tag="invq", name=f"invq{li}_{pb}"
                    )
                    # Exp(-0.5*ln(ss) + ln|T*w|) = |T*w| / sqrt(ss)
                    if wl is None:
                        nc.scalar.activation(invq[:], lnb[:], AF.Exp, scale=-0.5)
                    else:
                        nc.scalar.activation(
                            invq[:],
                            lnb[:],
                            AF.Exp,
                            scale=-0.5,
                            bias=w_col[:, wl : wl + 1],
                        )
                    invqs.append(invq)
                # k-outer so the first channel chunks are ready for the
                # main loop's first matmuls as early as possible
                for k in range(n_ci):
                    for pb in range(2):
                        sl = slice(k * P + pb * PB, k * P + pb * PB + PB)
                        nc.vector.tensor_mul(
                            dst[:, sl], raw[:, sl], invqs[pb][:]
                        )

            # layer 3 first: its chain is short, so the main loop's layer-3
            # matmuls (emitted first per chunk) start earliest; f_q is
            # normalized after the main loop (it is only needed in phase B).
            # P0 borrows the main PSUM banks that the main loop touches
            # late (ps4 via back(), ss4): the rotation deps are harmless.
            with ExitStack() as pctx:
                sqp = pctx.enter_context(tc.tile_pool(name="p0sq", bufs=2))
                mini = pctx.enter_context(tc.tile_pool(name="p0mini", bufs=2))
                stg = pctx.enter_context(tc.tile_pool(name="p0stg", bufs=1))
                q4s = stg.tile([128, NC4 * P], BF16)  # raw q4 staging
                q_norm_layer(
                    sqp, mini, mps, 0, qns3, qns3, NC3, q3, 1,
                    ss_tags=["ps4_0", "ps4_1"], ss_bufs=2,
                    bc_tag="ss4", bc_bufs=1,
                )
                q_norm_layer(
                    sqp, mini, mps, 1, q4s, qns4, NC4, q4, 0,
                    ss_tags=["ps4_0", "ps4_1"], ss_bufs=2,
                    bc_tag="ss4", bc_bufs=1,
                )

            # ------------- main: support stream, logits, exp -------------
            # Split per chunk into a front half (layer-3 logits, support
            # norms, exp3) and a back half (layer-4 logits, exp4, product).
            # Fronts run WARM chunks ahead so the PE has queued work while
            # P0's layer-4 scaling (a long DVE chain) finishes.
            WARM = 4
            NSCAL = 6  # chunks whose squares run on the scalar engine
            if True:
                qns4r = qns4[:].rearrange("c (ci p) -> c ci p", p=P)
                state = {}

                ones8r = ones8[:].rearrange("c (k f) -> c k f", f=32)

                def support_inv(qc, qn, tag, sq_t, n_ci, itag, ibufs):
                    """1/||s_q|| as a [q, 1] column: fp8 DoubleRow with the
                    ones-pair stationary and squared chunk-pairs streaming
                    (K=256 per row), row Ln/Exp, then a K=1 matmul transposes
                    the row back onto partitions (same PSUM bank, rotated)."""
                    ssr = mps.tile([32, 128], F32, tag=tag, name=f"{tag}r_{qc}")
                    sqr = sq_t[:].rearrange("c (ci q) -> c ci q", q=128)
                    for i in range(n_ci // 2):
                        nc.tensor.matmul(
                            ssr[0:32, 0:qn],
                            ones8r[:],
                            sqr[:, 2 * i : 2 * i + 2, 0:qn],
                            start=(i == 0),
                            stop=(i == n_ci // 2 - 1),
                            perf_mode=DR,
                        )
                    lnr = mmini.tile([1, 128], F32, tag=f"ln{tag}", name=f"ln{tag}_{qc}")
                    nc.scalar.activation(lnr[0:1, 0:qn], ssr[0:1, 0:qn], AF.Ln)
                    invr = mmini.tile([1, 128], F32, tag=f"iv{tag}", name=f"iv{tag}_{qc}")
                    nc.scalar.activation(
                        invr[0:1, 0:qn], lnr[0:1, 0:qn], AF.Exp, scale=-0.5
                    )
                    col = mps.tile([128, 1], F32, tag=tag, name=f"{tag}c_{qc}")
                    nc.tensor.matmul(
                        col[0:qn, :], invr[0:1, 0:qn], ones_row[:, 0:1]
                    )
                    inv = mmini.tile(
                        [128, 1], F32, tag=itag, name=f"{itag}_{qc}", bufs=ibufs
                    )
                    nc.scalar.copy(inv[0:qn, :], col[0:qn, :])
                    return inv

                def front(qc):
                    qn = 128 if qc < NQC - 1 else QT
                    sn3 = snpool.tile(
                        [128, NC3 * 128], BF16, tag="sn3", name=f"sn3_{qc}"
                    )
                    sn4 = snpool.tile(
                        [128, NC4 * 128], F8, tag="sn4", name=f"sn4_{qc}",
                        bufs=WARM + 1,
                    )
                    load_blocks(sn3, 128, 0, s3, qc * 128, qn, NC3, eng=nc.gpsimd)
                    load_blocks(sn4, 128, 0, s4, qc * 128, qn, NC4, eng=nc.gpsimd)
                    nc.gpsimd.dma_start(
                        vt_all[0:qn, qc * CV : (qc + 1) * CV],
                        vt[qc * 128 : qc * 128 + qn, :],
                    )
                    sq3 = sqpool.tile(
                        [128, NC3 * 128], F8, tag="sq3", name=f"sq3_{qc}"
                    )
                    if qc < NSCAL:
                        # scalar-engine square: the early chunks' layer-3
                        # norm chain must not queue behind P0's DVE work
                        nc.scalar.square(sq3[:], sn3[:])
                    else:
                        nc.vector.tensor_mul(sq3[:], sn3[:], sn3[:])
                    # sq4 is only needed in back(qc), WARM chunks later, so
                    # the DVE always has slack for it
                    sq4 = sqpool.tile(
                        [128, NC4 * 128], F8, tag="sq4", name=f"sq4_{qc}",
                        bufs=WARM + 1,
                    )
                    nc.vector.tensor_mul(sq4[:], sn4[:], sn4[:])

                    ps3 = [
                        mps.tile(
                            [128, PB], F32, tag=f"ps3_{pb}", name=f"ps3_{pb}_{qc}"
                        )
                        for pb in range(2)
                    ]
                    for k in range(NC3):
                        lhsT = sn3[:, k * 128 : k * 128 + qn]
                        for pb in range(2):
                            nc.tensor.matmul(
                                ps3[pb][0:qn, :],
                                lhsT,
                                qns3[:, k * P + pb * PB : k * P + pb * PB + PB],
                                start=(k == 0),
                                stop=(k == NC3 - 1),
                            )
                    inv3 = support_inv(qc, qn, "ss3", sq3, NC3, "inv3", 2)
                    e3s = []
                    for pb in range(2):
                        e3 = epool.tile(
                            [128, PB], BF16, tag=f"e3_{pb}", name=f"e3_{pb}_{qc}",
                            bufs=WARM + 1,
                        )
                        nc.scalar.activation(
                            e3[0:qn, :],
                            ps3[pb][0:qn, :],
                            AF.Exp,
                            scale=inv3[0:qn, :],
                        )
                        e3s.append(e3)
                    state[qc] = (qn, sn4, sq4, e3s)

                def back(qc):
                    qn, sn4, sq4, e3s = state.pop(qc)
                    ps4 = [
                        mps.tile(
                            [128, PB], F32, tag=f"ps4_{pb}", name=f"ps4_{pb}_{qc}",
                            bufs=2,
                        )
                        for pb in range(2)
                    ]
                    sn4r = sn4[:].rearrange("c (ci q) -> c ci q", q=128)
                    for i in range(NC4 // 2):
                        lhsT = sn4r[:, 2 * i : 2 * i + 2, 0:qn]
                        for pb in range(2):
                            nc.tensor.matmul(
                                ps4[pb][0:qn, :],
                                lhsT,
                                qns4r[
                                    :, 2 * i : 2 * i + 2,
                                    pb * PB : pb * PB + PB,
                                ],
                                start=(i == 0),
                                stop=(i == NC4 // 2 - 1),
                                perf_mode=DR,
                            )
                    inv4 = support_inv(qc, qn, "ss4", sq4, NC4, "inv4", 2)
                    for pb in range(2):
                        e4 = epool.tile(
                            [128, PB], BF16, tag=f"e4_{pb}", name=f"e4_{pb}_{qc}"
                        )
                        nc.scalar.activation(
                            e4[0:qn, :],
                            ps4[pb][0:qn, :],
                            AF.Exp,
                            scale=inv4[0:qn, :],
                        )
                        nc.vector.tensor_mul(
                            expT[0:qn, qc * P + pb * PB : qc * P + pb * PB + PB],
                            e4[0:qn, :],
                            e3s[pb][0:qn, :],
                        )

                # warmup: fronts for the first WARM chunks fill the PE while
                # P0's layer-4 scaling finishes; afterwards chunks run
                # monolithically (front+back) so the steady state has a
                # single wait point per iteration (protects the p-state ramp)
                for qc in range(WARM):
                    front(qc)
                for qc in range(WARM):
                    back(qc)
                for qc in range(WARM, NQC):
                    front(qc)
                    back(qc)
            mctx.close()

            fstg = octx.enter_context(tc.tile_pool(name="fstg", bufs=1))
            fqn = fstg.tile([128, NCV * P], BF16)  # normalized f_q (ci, p)

            # ---------------- phase B: attention-weighted values ----------------
            with ExitStack() as bctx:
                bps = bctx.enter_context(
                    tc.tile_pool(name="bps", bufs=1, space="PSUM")
                )
                bmini = bctx.enter_context(tc.tile_pool(name="bmini", bufs=1))
                bsq = bctx.enter_context(tc.tile_pool(name="bsq", bufs=2))
                batt = bctx.enter_context(tc.tile_pool(name="batt", bufs=1))
                bout = bctx.enter_context(tc.tile_pool(name="bout", bufs=2))

                ssy = [
                    bps.tile([1, PB], F32, tag=f"ssy{pb}", name=f"ssy{pb}")
                    for pb in range(2)
                ]
                dns = [
                    bps.tile([1, PB], F32, tag=f"dn{pb}", name=f"dn{pb}")
                    for pb in range(2)
                ]
                att_sb = {}
                inv_dn, raw_dn = [], []
                # denominators first: their matmuls keep the PE busy while
                # the f_q normalization chain (below) runs on DVE/scalar
                for qc in range(NQC):
                    for pb in range(2):
                        nc.tensor.matmul(
                            dns[pb][:],
                            ones_col[:],
                            expT[:, qc * P + pb * PB : qc * P + pb * PB + PB],
                            start=(qc == 0),
                            stop=(qc == NQC - 1),
                        )
                for pb in range(2):
                    u = bmini.tile([1, PB], F32, tag=f"ud{pb}", name=f"ud{pb}")
                    nc.scalar.copy(u[:], dns[pb][:])
                    bcp = bps.tile([128, PB], F32, tag=f"dn{pb}", name=f"bd{pb}")
                    nc.tensor.matmul(bcp[:], ones_row[:], u[:])
                    raw = bmini.tile(
                        [128, PB], F32, tag=f"dnraw{pb}", name=f"dnraw{pb}"
                    )
                    nc.scalar.copy(raw[:], bcp[:])
                    inv = bmini.tile(
                        [128, PB], F32, tag=f"dninv{pb}", name=f"dninv{pb}"
                    )
                    nc.vector.reciprocal(inv[:], bcp[:])
                    inv_dn.append(inv)
                    raw_dn.append(raw)

                # f_q normalization: deferred to here (only the phase-B
                # epilogue reads fqn) so its DVE/scalar chain hides under
                # the dn/Y matmuls; borrows the y0 PSUM banks (their first
                # real use comes later, the rotation deps are harmless)
                with ExitStack() as fctx:
                    fsqp = fctx.enter_context(tc.tile_pool(name="fsq", bufs=2))
                    fmini = fctx.enter_context(tc.tile_pool(name="fmini", bufs=2))
                    q_norm_layer(
                        fsqp, fmini, bps, 2, fqn, fqn, NCV, fq, None,
                        ss_tags=["y0_0", "y0_1"], ss_bufs=1,
                        bc_tag="y1_0", bc_bufs=1,
                    )
                for pss in range(2):
                    ys = {}
                    for cbk in range(2):
                        cb = 2 * pss + cbk
                        for pb in range(2):
                            ys[(cb, pb)] = bps.tile(
                                [128, PB], F32, tag=f"y{cbk}_{pb}",
                                name=f"y{cb}_{pb}",
                            )
                    for qc in range(NQC):
                        for cbk in range(2):
                            cb = 2 * pss + cbk
                            lhsT = vt_all[
                                :, qc * CV + cb * 128 : qc * CV + (cb + 1) * 128
                            ]
                            for pb in range(2):
                                nc.tensor.matmul(
                                    ys[(cb, pb)][:],
                                    lhsT,
                                    expT[:, qc * P + pb * PB : qc * P + pb * PB + PB],
                                    start=(qc == 0),
                                    stop=(qc == NQC - 1),
                                )
                    # pb-major drains: ssy[pb0] stops as early as possible so
                    # the pb0 epilogue overlaps the pb1 drains of pass 1
                    for pb in range(2):
                        for cbk in range(2):
                            cb = 2 * pss + cbk
                            att = batt.tile(
                                [128, PB], F32, tag=f"att{cb}_{pb}",
                                name=f"att{cb}_{pb}",
                            )
                            nc.vector.tensor_mul(
                                att[:], ys[(cb, pb)][:], inv_dn[pb][:]
                            )
                            att_sb[(cb, pb)] = att
                            nc.sync.dma_start(
                                att_o[
                                    cb * 128 : (cb + 1) * 128,
                                    pb * PB : (pb + 1) * PB,
                                ],
                                att[:],
                            )
                            sqy = bsq.tile([128, PB], BF16, tag="sqy")
                            nc.scalar.square(sqy[:], ys[(cb, pb)][:])
                            nc.tensor.matmul(
                                ssy[pb][:],
                                ones_col[:],
                                sqy[:],
                                start=(cb == 0),
                                stop=(cb == NCV - 1),
                            )
                        if pss == 1:
                            # per-pb epilogue right after ssy[pb] stops
                            u = bmini.tile(
                                [1, PB], F32, tag=f"us{pb}", name=f"us{pb}"
                            )
                            nc.scalar.copy(u[:], ssy[pb][:])
                            bcp = bps.tile(
                                [128, PB], F32, tag=f"ssy{pb}", name=f"bs{pb}"
                            )
                            nc.tensor.matmul(bcp[:], ones_row[:], u[:])
                            lnb = bmini.tile(
                                [128, PB], F32, tag=f"lnbs{pb}", name=f"lnbs{pb}"
                            )
                            nc.scalar.activation(lnb[:], bcp[:], AF.Ln)
                            # Exp(-0.5*ln(ssy) + ln(0.3)) = 0.3/||Y||
                            sinv = bmini.tile(
                                [128, PB], F32, tag=f"sinv{pb}", name=f"sinv{pb}"
                            )
                            nc.scalar.activation(
                                sinv[:], lnb[:], AF.Exp, scale=-0.5,
                                bias=lnw_att[:],
                            )
                            # fq = fqn + att * (denom * 0.3/||Y||)
                            s2 = bmini.tile(
                                [128, PB], F32, tag=f"s2{pb}", name=f"s2{pb}"
                            )
                            nc.vector.tensor_mul(s2[:], raw_dn[pb][:], sinv[:])
                            for cb in range(NCV):
                                t = bout.tile([128, PB], F32, tag="t")
                                nc.vector.tensor_mul(
                                    t[:], att_sb[(cb, pb)][:], s2[:]
                                )
                                f_sb = bout.tile([128, PB], F32, tag="f")
                                nc.vector.tensor_add(
                                    f_sb[:],
                                    t[:],
                                    fqn[
                                        :,
                                        cb * P + pb * PB : cb * P + pb * PB + PB,
                                    ],
                                )
                                # fq_o rides the gpsimd DMA queue so the two
                                # output streams drain in parallel
                                nc.gpsimd.dma_start(
                                    fq_o[
                                        cb * 128 : (cb + 1) * 128,
                                        pb * PB : (pb + 1) * PB,
                                    ],
                                    f_sb[:],
                                )
    _split_sync_waits(nc)
    return nc


def make_in_maps(fq_l3, fs_l3, fq_l4, fs_l4, f_q, f_s, w_red):
    bf = ml_dtypes.bfloat16
    wr = np.asarray(w_red, np.float32)
    wvec = np.log(np.abs(TEMP * wr)).reshape(1, 2)
    # fold sign(w_l) into the query features; |T*w_l| rides in wvec
    q4f = float(np.sign(wr[0])) * np.asarray(fq_l4, np.float32).reshape(B, C4, HW)
    q3f = float(np.sign(wr[1])) * np.asarray(fq_l3, np.float32).reshape(B, C3, HW)
    s4f = np.asarray(fs_l4, np.float32).reshape(B, C4, HW)
    s3f = np.asarray(fs_l3, np.float32).reshape(B, C3, HW)
    vf = np.asarray(f_s, np.float32).reshape(B, CV, HW)
    fqf = np.asarray(f_q, np.float32).reshape(B, CV, HW)
    f8 = ml_dtypes.float8_e4m3
    s4b = [np.ascontiguousarray(s4f[b]).astype(bf) for b in range(B)]
    s48 = [x.astype(f8) for x in s4b]
    s3b = [np.ascontiguousarray(s3f[b]).astype(bf) for b in range(B)]
    vtb = [np.ascontiguousarray(vf[b].T).astype(bf) for b in range(B)]
    in_maps = []
    for k in range(NCORES):
        b, j = divmod(k, PSH)
        sl = slice(j * P, (j + 1) * P)
        in_maps.append(
            {
                "q4": np.ascontiguousarray(q4f[b][:, sl]).astype(bf),
                "q3": np.ascontiguousarray(q3f[b][:, sl]).astype(bf),
                "s4": s48[b],
                "s3": s3b[b],
                "vt": vtb[b],
                "fq": np.ascontiguousarray(fqf[b][:, sl]).astype(bf),
                "wv": np.ascontiguousarray(wvec, np.float32),
            }
        )
    return in_maps


def gather_outputs(results):
    att = np.empty((B, CV, HW), np.float32)
    fqo = np.empty((B, CV, HW), np.float32)
    for k in range(NCORES):
        b, j = divmod(k, PSH)
        sl = slice(j * P, (j + 1) * P)
        att[b][:, sl] = results[k]["att_o"]
        fqo[b][:, sl] = results[k]["fq_o"]
    return (
        fqo.reshape(B, CV, H, W),
        att.reshape(B, CV, H, W),
    )


def kernel(fq_l3, fs_l3, fq_l4, fs_l4, f_q, f_s, w_red, trace=False):
    nc = build()
    in_maps = make_in_maps(fq_l3, fs_l3, fq_l4, fs_l4, f_q, f_s, w_red)
    res = run_bass_kernel_spmd(nc, in_maps, core_ids=list(range(NCORES)), trace=trace)
    out = gather_outputs(res.results)
    if trace:
        return out, res
    return out
